# revision 45
# baseline (speedup 1.0000x reference)
"""DiffAttn kernel for 8 trn2 NeuronCores — v2 (component-split, fp16).

Problem (per reference):
  X [4, 4096, 1024]; Wq/Wk [1024, 256]; Wv [1024, 128]; biases; lam scalar.
  Q,K = X@Wq+bq, X@Wk+bk ; V = X@Wv+bv
  A_i = Q_i @ K_i^T / sqrt(128)  (i = 1,2 : the two 128-wide halves)
  out = (softmax(A1) - lam * softmax(A2)) @ V          -> [4, 4096, 128]

Sharding: 8 cores = 4 batches x 2 softmax components. Core (b, i) computes
n_i = softmax(A_i) @ V for ALL 4096 queries of batch b, normalized and (for
i=1) pre-scaled by lam via the broadcast matmul; the host computes
out[b] = n_0 - n_1 (pure elementwise post-processing, like the transpose).
This removes the K-projection redundancy of a query-split (each core
projects only its component's Q_i/K_i; V stays duplicated per pair).

All matmul data is fp16 (host converts X/W); PSUM accumulates fp32.
Per-core dataflow:
  XT [1024, 4096] fp16 streamed in 512-col chunks; Q_i/K_i projected into
  head-transposed layout [128, S]; V via DMA-xbar transpose into [key, d]
  blocks (no PE transpose, no PSUM). Bias-adds ride the PSUM->SBUF
  evacuation on VectorE (fp16 out). Scores S^T[sk, sq] per 1024-query
  super-chunk; exp on ScalarE (1/sqrt(D) folded into activation scale)
  writing fp16; softmax denominators accumulated on VectorE in fp16 (2x
  DVE mode); PV accumulated over the 32 key tiles in PSUM. Finalize:
  denominator row via fp16 ones-matmul column-sum, reciprocal via the
  fast custom-DVE approx (no ACT table switch), broadcast across
  partitions by a K=1 f32r matmul with lam folded into the stationary
  row, one fp16 multiply, ship O^T[128, 4096] fp16.

Emission interleaves the remaining projection chunks into super 0 and
runs the next super's scores/exp/denominator ahead (PV deferred until the
single o-PSUM accumulator frees) so ScalarE never starves while PE works
through projections.
"""

import os
import sys

sys.path.insert(0, "/opt/trn_rl_repo")

import numpy as np

import concourse.bacc as bacc
import concourse.mybir as mybir
from concourse.tile import TileContext
from concourse.bass_utils import run_bass_kernel_spmd

F32 = mybir.dt.float32
F16 = mybir.dt.float16
F32R = mybir.dt.float32r
AF = mybir.ActivationFunctionType

D = 128
EMB = 1024
B, S = 4, 4096
SQC = 512            # projection column chunk
NCC = S // SQC       # 8 chunks
NE = EMB // 128      # 8 emb tiles
SUP = 1024           # attention query super-chunk
NSUP = S // SUP      # 4
NSK = S // 128       # 32 key tiles
RA0 = 24             # run-ahead iters of super 1 emitted during super 0
RAL = 8              # boundary run-ahead for later supers
INV_SQRT_D = 1.0 / np.sqrt(np.float32(D))

TRACE = False
TRACE_DIR = None
LAST_RESULT = None


def _build():
    nc = bacc.Bacc("TRN2", target_bir_lowering=False, debug=False, num_devices=8)

    xt = nc.dram_tensor("xt", [EMB, S], F16, kind="ExternalInput")
    wq = nc.dram_tensor("wq", [EMB, D], F16, kind="ExternalInput")
    wk = nc.dram_tensor("wk", [EMB, D], F16, kind="ExternalInput")
    wv = nc.dram_tensor("wv", [EMB, D], F16, kind="ExternalInput")
    bq = nc.dram_tensor("bq", [D, 1], F32, kind="ExternalInput")
    bk = nc.dram_tensor("bk", [D, 1], F32, kind="ExternalInput")
    bv = nc.dram_tensor("bv", [D, 1], F32, kind="ExternalInput")
    lam_row = nc.dram_tensor("lam_row", [1, 128], F16, kind="ExternalInput")
    out = nc.dram_tensor("o", [D, S], F16, kind="ExternalOutput")  # O_i^T / r

    from contextlib import ExitStack

    with TileContext(nc) as tc, ExitStack() as ctx:
        xpool = ctx.enter_context(tc.tile_pool(name="xt", bufs=6))
        cpool = ctx.enter_context(tc.tile_pool(name="const", bufs=1))
        wpool = ctx.enter_context(tc.tile_pool(name="w", bufs=1))
        qkv = ctx.enter_context(tc.tile_pool(name="qkv", bufs=1))
        vpool = ctx.enter_context(tc.tile_pool(name="vts", bufs=2))
        epool = ctx.enter_context(tc.tile_pool(name="e", bufs=28))
        apool = ctx.enter_context(tc.tile_pool(name="acc", bufs=3))
        fpool = ctx.enter_context(tc.tile_pool(name="fin", bufs=2))
        smpool = ctx.enter_context(tc.tile_pool(name="small", bufs=2))
        pmm = ctx.enter_context(tc.tile_pool(name="pmm", bufs=1, space="PSUM"))

        # constants / small inputs
        ones_col = cpool.tile([128, 1], F16, tag="ones_col")
        nc.vector.memset(ones_col[:], 1.0)
        jsrc = cpool.tile([128, 512], F16, tag="jsrc")
        nc.vector.memset(jsrc[:], 0.0)
        bq_t = cpool.tile([128, 1], F32, tag="bq")
        bk_t = cpool.tile([128, 1], F32, tag="bk")
        bv_t = cpool.tile([128, 1], F32, tag="bv")
        lam_t = cpool.tile([1, 128], F16, tag="lam")
        nc.gpsimd.dma_start(out=bq_t[:], in_=bq[:, :])
        nc.gpsimd.dma_start(out=bk_t[:], in_=bk[:, :])
        nc.gpsimd.dma_start(out=bv_t[:], in_=bv[:, :])
        nc.gpsimd.dma_start(out=lam_t[:], in_=lam_row[:, :])

        # weights [128, NE, 128] fp16 — one DMA each (issue slots are precious)
        wq_t = wpool.tile([128, NE, 128], F16, tag="wq")
        wk_t = wpool.tile([128, NE, 128], F16, tag="wk")
        wv_t = wpool.tile([128, NE, 128], F16, tag="wv")

        def wsrc(w):
            return w[:, :].rearrange("(t p) d -> p t d", p=128)



        xts = {}

        def load_chunk(cc, sliced=False):
            # chunks split across the two HWDGE queues; prologue chunks are
            # e-tile-sliced so projections can start on slice 0 (subtile deps)
            if cc in xts or cc >= NCC:
                return
            t = xpool.tile([128, NE, SQC], F16, tag="xchunk", name=f"xc_{cc}")
            c0 = cc * SQC
            if sliced:
                for j, eng in ((0, nc.scalar), (1, nc.sync), (2, nc.scalar), (3, nc.sync)):
                    r = slice(j * 256, (j + 1) * 256)
                    eng.dma_start(
                        out=t[:, 2 * j : 2 * j + 2, :],
                        in_=xt[r, c0 : c0 + SQC].rearrange(
                            "(t p) s -> p t s", p=128
                        ),
                    )
            else:
                h = SQC // 2
                nc.scalar.dma_start(
                    out=t[:, :, 0:h],
                    in_=xt[:, c0 : c0 + h].rearrange("(t p) s -> p t s", p=128),
                )
                nc.sync.dma_start(
                    out=t[:, :, h:SQC],
                    in_=xt[:, c0 + h : c0 + SQC].rearrange("(t p) s -> p t s", p=128),
                )
            xts[cc] = t

        # projected tensors
        qt = qkv.tile([128, S], F16, tag="qt")   # Q_i^T
        kt = qkv.tile([128, S], F16, tag="kt")   # K_i^T
        vv = qkv.tile([128, S], F16, tag="vv")   # col blk*128+j : V[key, d]

        # ---------------- projection of one chunk ----------------
        # emitted in half-group pieces (4 matmuls each) so the in-order PE
        # stream never starves ScalarE for a whole chunk
        pend_tr = []

        def proj_pieces(cc):
            csl = slice(cc * SQC, (cc + 1) * SQC)
            state = {}
            pieces = []

            def mk(dst, w_t, b_t, gtag):
                def p1():
                    ps = pmm.tile(
                        [128, SQC], F32, tag="s", bufs=3, name=f"pp_{gtag}_{cc}"
                    )
                    state[gtag] = ps
                    for t in range(NE // 2):
                        nc.tensor.matmul(
                            ps[:], w_t[:, t, :], xts[cc][:, t, :],
                            start=(t == 0), stop=False,
                        )

                def p2():
                    ps = state[gtag]
                    for t in range(NE // 2, NE):
                        nc.tensor.matmul(
                            ps[:], w_t[:, t, :], xts[cc][:, t, :],
                            start=False, stop=(t == NE - 1),
                        )
                    if dst is not None:
                        nc.vector.tensor_scalar_add(dst[:, csl], ps[:], b_t[:, 0:1])
                    else:
                        vt_s = vpool.tile(
                            [128, SQC], F16, tag="vts", name=f"vts_{cc}"
                        )
                        nc.vector.tensor_scalar_add(vt_s[:], ps[:], b_t[:, 0:1])

                        def tr(cc=cc, vt_s=vt_s):
                            for j in range(SQC // 128):
                                col = (cc * (SQC // 128) + j) * 128
                                nc.sync.dma_start_transpose(
                                    vv[:, col : col + 128],
                                    vt_s[:, j * 128 : (j + 1) * 128],
                                )

                        # transposes ride the Sync queue but are emitted
                        # deferred so prologue chunk loads aren't blocked
                        if pend_tr and pend_tr[0] is None:
                            tr()
                        else:
                            pend_tr.append(tr)

                pieces.extend([p1, p2])

            mk(kt, wk_t, bk_t, "k")
            mk(qt, wq_t, bq_t, "q")
            mk(None, wv_t, bv_t, "v")
            return pieces

        def proj(cc):
            for p in proj_pieces(cc):
                p()

        # ---------------- attention ----------------
        st = {}

        def attn_state(c):
            st[c] = dict(
                o=None,
                pacc=apool.tile([128, SUP], F16, tag="pacc", name=f"pacc_{c}"),
                e={},
            )

        def attn_sqexp(c, skt):
            """scores + exp + denominator-accumulate for (c, skt)."""
            s = st[c]
            ksl = slice(skt * 128, (skt + 1) * 128)
            s_ps = pmm.tile([128, SUP], F32, tag="s", bufs=3, name=f"s_{c}_{skt}")
            for h in range(2):
                hsl = slice(h * 512, (h + 1) * 512)
                nc.tensor.matmul(
                    s_ps[:, hsl], kt[:, ksl],
                    qt[:, c * SUP + h * 512 : c * SUP + (h + 1) * 512],
                    start=True, stop=True,
                )
            e_t = epool.tile([128, SUP], F16, tag="e", name=f"e_{c}_{skt}")
            nc.scalar.activation(e_t[:], s_ps[:], AF.Exp, scale=float(INV_SQRT_D))
            pacc = s["pacc"]
            if skt == 0:
                nc.vector.tensor_copy(pacc[:], e_t[:])
            else:
                nc.vector.tensor_add(pacc[:], pacc[:], e_t[:])
            s["e"][skt] = e_t

        def attn_pv(c, skt):
            s = st[c]
            if s["o"] is None:
                s["o"] = pmm.tile([128, SUP], F32, tag="o", bufs=1, name=f"o_{c}")
            ksl = slice(skt * 128, (skt + 1) * 128)
            e_t = s["e"].pop(skt)
            for h in range(2):
                hsl = slice(h * 512, (h + 1) * 512)
                nc.tensor.matmul(
                    s["o"][:, hsl], vv[:, ksl], e_t[:, hsl],
                    start=(skt == 0), stop=(skt == NSK - 1),
                )

        fin_ib = {}

        def finalize_r(c):
            """Denominator chain — depends only on pacc, overlaps the PV drain."""
            s = st[c]
            rs = pmm.tile([1, SUP], F32, tag="s", bufs=3, name=f"rs_{c}")
            for h in range(2):
                hsl = slice(h * 512, (h + 1) * 512)
                nc.tensor.matmul(
                    rs[0:1, hsl], ones_col[:], s["pacc"][:, hsl],
                    start=True, stop=True,
                )
            r_inv = smpool.tile([1, SUP], F32, tag="rinv", name=f"rinv_{c}")
            nc.vector.reciprocal_approx_fast(r_inv[0:1, :], rs[0:1, :])
            r16 = smpool.tile([1, SUP], F16, tag="r16", name=f"r16_{c}")
            nc.vector.tensor_copy(r16[0:1, :], r_inv[0:1, :])
            # broadcast lam/r across partitions (lam folded into stationary row)
            ib = pmm.tile([128, SUP], F32, tag="s", bufs=3, name=f"ib_{c}")
            for h in range(2):
                hsl = slice(h * 512, (h + 1) * 512)
                nc.tensor.matmul(
                    ib[:, hsl], lam_t[0:1, :], r16[0:1, hsl],
                    start=True, stop=True,
                )
            fin_ib[c] = ib

        def finalize_o(c):
            s = st.pop(c)
            # evacuate O on ScalarE (idle at boundaries) so the DVE chain
            # (recip/r16) runs in parallel; frees the o psum for the next super
            o_s = fpool.tile([128, SUP], F16, tag="os", name=f"os_{c}")
            nc.scalar.copy(o_s[:], s["o"][:])
            o_t = fpool.tile([128, SUP], F16, tag="ot", name=f"ot_{c}")
            nc.vector.tensor_mul(o_t[:], o_s[:], fin_ib.pop(c)[:])
            nc.sync.dma_start(out=out[:, c * SUP : (c + 1) * SUP], in_=o_t[:])

        # ---------------- emission schedule ----------------
        # prologue DMA order tuned so proj(0)'s first matmul has wk slice 0 +
        # chunk-0 slice 0 within ~2us of queue start
        nc.sync.dma_start(out=wk_t[:, 0:4, :], in_=wk[0:512, :].rearrange("(t p) d -> p t d", p=128))
        load_chunk(0, sliced=True)
        nc.sync.dma_start(out=wk_t[:, 4:8, :], in_=wk[512:1024, :].rearrange("(t p) d -> p t d", p=128))
        nc.scalar.dma_start(out=wq_t[:], in_=wsrc(wq))
        nc.sync.dma_start(out=wv_t[:], in_=wsrc(wv))
        load_chunk(1, sliced=True)
        load_chunk(2)
        load_chunk(3)
        load_chunk(4)
        load_chunk(5)
        # junk matmuls against the HAM clock gate: keep the PE busy while the
        # first chunk streams in so projections start at 2.4 GHz
        junk = pmm.tile([1, 512], F32, tag="o", bufs=1, name="junk")
        for _ in range(6):
            nc.tensor.matmul(junk[0:1, :], ones_col[:], jsrc[:], start=True, stop=True)
        proj(0)
        load_chunk(6)
        proj(1)
        load_chunk(7)
        for t_fn in pend_tr:
            t_fn()
        pend_tr.clear()
        pend_tr.append(None)  # sentinel: emit transposes inline from now on
        # remaining proj chunks drip into super 0 as half-group pieces;
        # chunk cc must complete before attn(0, 4*cc)
        piece_q = []
        for cc in range(2, NCC):
            piece_q.extend(proj_pieces(cc))

        # ra_in[c]: iterations of super c pre-run during super c-1.  Super 0's
        # window is PE-bound (projections), so most of super 1 pre-runs there
        # to keep ScalarE saturated; later boundaries only need a small lap.
        ra_in = [0, RA0, RAL, RAL]

        def runahead_plan(c):
            """(skt -> list of next-super iters to emit) during super c."""
            if c + 1 >= NSUP:
                return {}
            nxt = ra_in[c + 1]
            plan = {}
            if c == 0:
                # gated on proj(3) (queries of super 1) at emission slot 9
                plan[10] = [0, 1, 2]
                for skt in range(11, 11 + nxt - 3):
                    plan[skt] = [skt - 8]
            else:
                fresh = list(range(ra_in[c], NSK))
                for j in range(nxt):
                    skt = fresh[min(len(fresh) - 1, (j * len(fresh)) // nxt)]
                    plan.setdefault(skt, []).append(j)
            return plan

        attn_state(0)
        next_piece = [0]
        for c in range(NSUP):
            rb = ra_in[c]
            plan = runahead_plan(c)
            pvq = 0
            fresh = list(range(rb, NSK))
            for idx, skt in enumerate(fresh):
                attn_sqexp(c, skt)
                # previous super's finalize lands after this super's first two
                # score emissions so rs/ib don't choke the psum ring and the
                # engines finalize in parallel with fresh work
                if c > 0 and idx == 1:
                    finalize_r(c - 1)
                    finalize_o(c - 1)
                # PV lags scores >= 1 iter (in-order PE stream must not head-
                # block on exp); super c's PVs wait for finalize_o(c-1) to
                # free the o accumulator; catch-up drains a few per iter.
                if c == 0 or idx >= 2:
                    budget = min(4, max(1, -(-(NSK - pvq) // max(1, NSK - skt))))
                    for _ in range(budget):
                        if pvq <= skt - 1:
                            attn_pv(c, pvq)
                            pvq += 1
                if c == 0:
                    target = min(len(piece_q), ((skt + 1) * 4 + 2) // 3)
                    while next_piece[0] < target:
                        piece_q[next_piece[0]]()
                        next_piece[0] += 1
                for na in plan.get(skt, ()):
                    if na == 0:
                        attn_state(c + 1)
                    attn_sqexp(c + 1, na)
            while pvq < NSK:
                attn_pv(c, pvq)
                pvq += 1
        finalize_r(NSUP - 1)
        finalize_o(NSUP - 1)

    nc.compile()
    return nc


_NC = None


def _get_nc():
    global _NC
    if _NC is None:
        _NC = _build()
    return _NC


def kernel(X, lam, Wq, bq, Wk, bk, Wv, bv):
    X = np.asarray(X, dtype=np.float32)
    lam_f = float(np.asarray(lam))
    Wq = np.asarray(Wq, np.float32)
    Wk = np.asarray(Wk, np.float32)
    Wv = np.asarray(Wv, np.float32)
    bq_a = np.asarray(bq, np.float32).reshape(2 * D, 1)
    bk_a = np.asarray(bk, np.float32).reshape(2 * D, 1)
    bv_a = np.asarray(bv, np.float32).reshape(D, 1).copy()
    wv16 = np.ascontiguousarray(Wv.astype(np.float16))

    nc = _get_nc()

    in_maps = []
    for core in range(8):
        b, i = divmod(core, 2)
        dsl = slice(i * D, (i + 1) * D)
        in_maps.append(
            {
                "xt": np.ascontiguousarray(X[b].T.astype(np.float16)),
                "wq": np.ascontiguousarray(Wq[:, dsl].astype(np.float16)),
                "wk": np.ascontiguousarray(Wk[:, dsl].astype(np.float16)),
                "wv": wv16,
                "bq": np.ascontiguousarray(bq_a[dsl]),
                "bk": np.ascontiguousarray(bk_a[dsl]),
                "bv": bv_a,
                "lam_row": np.full((1, 128), 1.0 if i == 0 else lam_f, np.float16),
            }
        )

    global LAST_RESULT
    kwargs = {}
    if TRACE:
        import tempfile

        tdir = tempfile.mkdtemp(dir=TRACE_DIR) if TRACE_DIR else None
        kwargs = dict(trace=True, tmpdir=tdir)
    res = run_bass_kernel_spmd(nc, in_maps, list(range(8)), **kwargs)
    LAST_RESULT = res

    o = np.empty((B, S, D), np.float32)
    for b in range(B):
        n0 = res.results[2 * b]["o"].astype(np.float32)
        n1 = res.results[2 * b + 1]["o"].astype(np.float32)
        o[b] = (n0 - n1).T
    return o


# revision 48
# speedup vs baseline: 1.0026x; 1.0026x over previous
"""DiffAttn kernel for 8 trn2 NeuronCores — v2 (component-split, fp16).

Problem (per reference):
  X [4, 4096, 1024]; Wq/Wk [1024, 256]; Wv [1024, 128]; biases; lam scalar.
  Q,K = X@Wq+bq, X@Wk+bk ; V = X@Wv+bv
  A_i = Q_i @ K_i^T / sqrt(128)  (i = 1,2 : the two 128-wide halves)
  out = (softmax(A1) - lam * softmax(A2)) @ V          -> [4, 4096, 128]

Sharding: 8 cores = 4 batches x 2 softmax components. Core (b, i) computes
n_i = softmax(A_i) @ V for ALL 4096 queries of batch b, normalized and (for
i=1) pre-scaled by lam via the broadcast matmul; the host computes
out[b] = n_0 - n_1 (pure elementwise post-processing, like the transpose).
This removes the K-projection redundancy of a query-split (each core
projects only its component's Q_i/K_i; V stays duplicated per pair).

All matmul data is fp16 (host converts X/W); PSUM accumulates fp32.
Per-core dataflow:
  XT [1024, 4096] fp16 streamed in 512-col chunks; Q_i/K_i projected into
  head-transposed layout [128, S]; V via DMA-xbar transpose into [key, d]
  blocks (no PE transpose, no PSUM). Bias-adds ride the PSUM->SBUF
  evacuation on VectorE (fp16 out). Scores S^T[sk, sq] per 1024-query
  super-chunk; exp on ScalarE (1/sqrt(D) folded into activation scale)
  writing fp16; softmax denominators accumulated on VectorE in fp16 (2x
  DVE mode); PV accumulated over the 32 key tiles in PSUM. Finalize:
  denominator row via fp16 ones-matmul column-sum, reciprocal via the
  fast custom-DVE approx (no ACT table switch), broadcast across
  partitions by a K=1 f32r matmul with lam folded into the stationary
  row, one fp16 multiply, ship O^T[128, 4096] fp16.

Emission interleaves the remaining projection chunks into super 0 and
runs the next super's scores/exp/denominator ahead (PV deferred until the
single o-PSUM accumulator frees) so ScalarE never starves while PE works
through projections.
"""

import os
import sys

sys.path.insert(0, "/opt/trn_rl_repo")

import numpy as np

import concourse.bacc as bacc
import concourse.mybir as mybir
from concourse.tile import TileContext
from concourse.bass_utils import run_bass_kernel_spmd

F32 = mybir.dt.float32
F16 = mybir.dt.float16
F32R = mybir.dt.float32r
AF = mybir.ActivationFunctionType

D = 128
EMB = 1024
B, S = 4, 4096
SQC = 512            # projection column chunk
NCC = S // SQC       # 8 chunks
NE = EMB // 128      # 8 emb tiles
SUP = 1024           # attention query super-chunk
NSUP = S // SUP      # 4
NSK = S // 128       # 32 key tiles
RA0 = 24             # run-ahead iters of super 1 emitted during super 0
RAL = 8              # boundary run-ahead for later supers
INV_SQRT_D = 1.0 / np.sqrt(np.float32(D))

TRACE = False
TRACE_DIR = None
LAST_RESULT = None


def _build():
    nc = bacc.Bacc("TRN2", target_bir_lowering=False, debug=False, num_devices=8)

    xt = nc.dram_tensor("xt", [EMB, S], F16, kind="ExternalInput")
    wq = nc.dram_tensor("wq", [EMB, D], F16, kind="ExternalInput")
    wk = nc.dram_tensor("wk", [EMB, D], F16, kind="ExternalInput")
    wv = nc.dram_tensor("wv", [EMB, D], F16, kind="ExternalInput")
    bq = nc.dram_tensor("bq", [D, 1], F32, kind="ExternalInput")
    bk = nc.dram_tensor("bk", [D, 1], F32, kind="ExternalInput")
    bv = nc.dram_tensor("bv", [D, 1], F32, kind="ExternalInput")
    lam_row = nc.dram_tensor("lam_row", [1, 128], F16, kind="ExternalInput")
    out = nc.dram_tensor("o", [D, S], F16, kind="ExternalOutput")  # O_i^T / r

    from contextlib import ExitStack

    with TileContext(nc) as tc, ExitStack() as ctx:
        xpool = ctx.enter_context(tc.tile_pool(name="xt", bufs=6))
        cpool = ctx.enter_context(tc.tile_pool(name="const", bufs=1))
        wpool = ctx.enter_context(tc.tile_pool(name="w", bufs=1))
        qkv = ctx.enter_context(tc.tile_pool(name="qkv", bufs=1))
        vpool = ctx.enter_context(tc.tile_pool(name="vts", bufs=2))
        epool = ctx.enter_context(tc.tile_pool(name="e", bufs=28))
        apool = ctx.enter_context(tc.tile_pool(name="acc", bufs=3))
        fpool = ctx.enter_context(tc.tile_pool(name="fin", bufs=2))
        smpool = ctx.enter_context(tc.tile_pool(name="small", bufs=2))
        pmm = ctx.enter_context(tc.tile_pool(name="pmm", bufs=1, space="PSUM"))

        # constants / small inputs
        ones_col = cpool.tile([128, 1], F16, tag="ones_col")
        nc.vector.memset(ones_col[:], 1.0)
        jsrc = cpool.tile([128, 512], F16, tag="jsrc")
        nc.vector.memset(jsrc[:], 0.0)
        from concourse import masks

        ident = cpool.tile([128, 128], F16, tag="ident")
        masks.make_identity(nc, ident[:])
        bq_t = cpool.tile([128, 1], F32, tag="bq")
        bk_t = cpool.tile([128, 1], F32, tag="bk")
        bv_t = cpool.tile([128, 1], F32, tag="bv")
        lam_t = cpool.tile([1, 128], F16, tag="lam")
        nc.gpsimd.dma_start(out=bq_t[:], in_=bq[:, :])
        nc.gpsimd.dma_start(out=bk_t[:], in_=bk[:, :])
        nc.gpsimd.dma_start(out=bv_t[:], in_=bv[:, :])
        nc.gpsimd.dma_start(out=lam_t[:], in_=lam_row[:, :])

        # weights [128, NE, 128] fp16 — one DMA each (issue slots are precious)
        wq_t = wpool.tile([128, NE, 128], F16, tag="wq")
        wk_t = wpool.tile([128, NE, 128], F16, tag="wk")
        wv_t = wpool.tile([128, NE, 128], F16, tag="wv")

        def wsrc(w):
            return w[:, :].rearrange("(t p) d -> p t d", p=128)



        xts = {}

        def load_chunk(cc, sliced=False):
            # chunks split across the two HWDGE queues; prologue chunks are
            # e-tile-sliced so projections can start on slice 0 (subtile deps)
            if cc in xts or cc >= NCC:
                return
            t = xpool.tile([128, NE, SQC], F16, tag="xchunk", name=f"xc_{cc}")
            c0 = cc * SQC
            if sliced:
                for j, eng in ((0, nc.scalar), (1, nc.sync), (2, nc.scalar), (3, nc.sync)):
                    r = slice(j * 256, (j + 1) * 256)
                    eng.dma_start(
                        out=t[:, 2 * j : 2 * j + 2, :],
                        in_=xt[r, c0 : c0 + SQC].rearrange(
                            "(t p) s -> p t s", p=128
                        ),
                    )
            else:
                h = SQC // 2
                nc.scalar.dma_start(
                    out=t[:, :, 0:h],
                    in_=xt[:, c0 : c0 + h].rearrange("(t p) s -> p t s", p=128),
                )
                nc.sync.dma_start(
                    out=t[:, :, h:SQC],
                    in_=xt[:, c0 + h : c0 + SQC].rearrange("(t p) s -> p t s", p=128),
                )
            xts[cc] = t

        # projected tensors
        qt = qkv.tile([128, S], F16, tag="qt")   # Q_i^T
        kt = qkv.tile([128, S], F16, tag="kt")   # K_i^T
        vv = qkv.tile([128, S], F16, tag="vv")   # col blk*128+j : V[key, d]

        # ---------------- projection of one chunk ----------------
        # emitted in half-group pieces (4 matmuls each) so the in-order PE
        # stream never starves ScalarE for a whole chunk
        pend_tr = []

        def proj_pieces(cc):
            csl = slice(cc * SQC, (cc + 1) * SQC)
            state = {}
            pieces = []

            def mk(dst, w_t, b_t, gtag):
                def p1():
                    ps = pmm.tile(
                        [128, SQC], F32, tag="s", bufs=3, name=f"pp_{gtag}_{cc}"
                    )
                    state[gtag] = ps
                    for t in range(NE // 2):
                        nc.tensor.matmul(
                            ps[:], w_t[:, t, :], xts[cc][:, t, :],
                            start=(t == 0), stop=False,
                        )

                def p2():
                    ps = state[gtag]
                    for t in range(NE // 2, NE):
                        nc.tensor.matmul(
                            ps[:], w_t[:, t, :], xts[cc][:, t, :],
                            start=False, stop=(t == NE - 1),
                        )
                    if dst is not None:
                        nc.vector.tensor_scalar_add(dst[:, csl], ps[:], b_t[:, 0:1])
                    else:
                        vt_s = vpool.tile(
                            [128, SQC], F16, tag="vts", name=f"vts_{cc}"
                        )
                        nc.vector.tensor_scalar_add(vt_s[:], ps[:], b_t[:, 0:1])
                        for j in range(SQC // 128):
                            col = (cc * (SQC // 128) + j) * 128
                            tr = pmm.tile(
                                [128, 128], F16, tag="s", bufs=3,
                                name=f"tr_{cc}_{j}",
                            )
                            nc.tensor.transpose(
                                tr[:], vt_s[:, j * 128 : (j + 1) * 128], ident[:]
                            )
                            nc.vector.tensor_copy(vv[:, col : col + 128], tr[:])

                pieces.extend([p1, p2])

            mk(kt, wk_t, bk_t, "k")
            mk(qt, wq_t, bq_t, "q")
            mk(None, wv_t, bv_t, "v")
            return pieces

        def proj(cc):
            for p in proj_pieces(cc):
                p()

        # ---------------- attention ----------------
        st = {}

        def attn_state(c):
            st[c] = dict(
                o=None,
                pacc=apool.tile([128, SUP], F16, tag="pacc", name=f"pacc_{c}"),
                e={},
            )

        def attn_sqexp(c, skt):
            """scores + exp + denominator-accumulate for (c, skt)."""
            s = st[c]
            ksl = slice(skt * 128, (skt + 1) * 128)
            s_ps = pmm.tile([128, SUP], F32, tag="s", bufs=3, name=f"s_{c}_{skt}")
            for h in range(2):
                hsl = slice(h * 512, (h + 1) * 512)
                nc.tensor.matmul(
                    s_ps[:, hsl], kt[:, ksl],
                    qt[:, c * SUP + h * 512 : c * SUP + (h + 1) * 512],
                    start=True, stop=True,
                )
            e_t = epool.tile([128, SUP], F16, tag="e", name=f"e_{c}_{skt}")
            nc.scalar.activation(e_t[:], s_ps[:], AF.Exp, scale=float(INV_SQRT_D))
            pacc = s["pacc"]
            if skt == 0:
                nc.vector.tensor_copy(pacc[:], e_t[:])
            else:
                nc.vector.tensor_add(pacc[:], pacc[:], e_t[:])
            s["e"][skt] = e_t

        def attn_pv(c, skt):
            s = st[c]
            if s["o"] is None:
                s["o"] = pmm.tile([128, SUP], F32, tag="o", bufs=1, name=f"o_{c}")
            ksl = slice(skt * 128, (skt + 1) * 128)
            e_t = s["e"].pop(skt)
            for h in range(2):
                hsl = slice(h * 512, (h + 1) * 512)
                nc.tensor.matmul(
                    s["o"][:, hsl], vv[:, ksl], e_t[:, hsl],
                    start=(skt == 0), stop=(skt == NSK - 1),
                )

        fin_ib = {}

        def finalize_r(c):
            """Denominator chain — depends only on pacc, overlaps the PV drain."""
            s = st[c]
            rs = pmm.tile([1, SUP], F32, tag="s", bufs=3, name=f"rs_{c}")
            for h in range(2):
                hsl = slice(h * 512, (h + 1) * 512)
                nc.tensor.matmul(
                    rs[0:1, hsl], ones_col[:], s["pacc"][:, hsl],
                    start=True, stop=True,
                )
            r_inv = smpool.tile([1, SUP], F32, tag="rinv", name=f"rinv_{c}")
            nc.vector.reciprocal_approx_fast(r_inv[0:1, :], rs[0:1, :])
            r16 = smpool.tile([1, SUP], F16, tag="r16", name=f"r16_{c}")
            nc.vector.tensor_copy(r16[0:1, :], r_inv[0:1, :])
            # broadcast lam/r across partitions (lam folded into stationary row)
            ib = pmm.tile([128, SUP], F32, tag="s", bufs=3, name=f"ib_{c}")
            for h in range(2):
                hsl = slice(h * 512, (h + 1) * 512)
                nc.tensor.matmul(
                    ib[:, hsl], lam_t[0:1, :], r16[0:1, hsl],
                    start=True, stop=True,
                )
            fin_ib[c] = ib

        def finalize_o(c):
            s = st.pop(c)
            # evacuate O on ScalarE (idle at boundaries) so the DVE chain
            # (recip/r16) runs in parallel; frees the o psum for the next super
            o_s = fpool.tile([128, SUP], F16, tag="os", name=f"os_{c}")
            nc.scalar.copy(o_s[:], s["o"][:])
            o_t = fpool.tile([128, SUP], F16, tag="ot", name=f"ot_{c}")
            nc.vector.tensor_mul(o_t[:], o_s[:], fin_ib.pop(c)[:])
            nc.sync.dma_start(out=out[:, c * SUP : (c + 1) * SUP], in_=o_t[:])

        # ---------------- emission schedule ----------------
        # prologue DMA order tuned so proj(0)'s first matmul has wk slice 0 +
        # chunk-0 slice 0 within ~2us of queue start
        nc.sync.dma_start(out=wk_t[:, 0:4, :], in_=wk[0:512, :].rearrange("(t p) d -> p t d", p=128))
        load_chunk(0, sliced=True)
        nc.sync.dma_start(out=wk_t[:, 4:8, :], in_=wk[512:1024, :].rearrange("(t p) d -> p t d", p=128))
        nc.scalar.dma_start(out=wq_t[:], in_=wsrc(wq))
        nc.sync.dma_start(out=wv_t[:], in_=wsrc(wv))
        load_chunk(1, sliced=True)
        load_chunk(2)
        load_chunk(3)
        load_chunk(4)
        load_chunk(5)
        # junk matmuls against the HAM clock gate: keep the PE busy while the
        # first chunk streams in so projections start at 2.4 GHz
        junk = pmm.tile([1, 512], F32, tag="o", bufs=1, name="junk")
        for _ in range(6):
            nc.tensor.matmul(junk[0:1, :], ones_col[:], jsrc[:], start=True, stop=True)
        proj(0)
        load_chunk(6)
        proj(1)
        load_chunk(7)
        # remaining proj chunks drip into super 0 as half-group pieces;
        # chunk cc must complete before attn(0, 4*cc)
        piece_q = []
        for cc in range(2, NCC):
            piece_q.extend(proj_pieces(cc))

        # ra_in[c]: iterations of super c pre-run during super c-1.  Super 0's
        # window is PE-bound (projections), so most of super 1 pre-runs there
        # to keep ScalarE saturated; later boundaries only need a small lap.
        ra_in = [0, RA0, RAL, RAL]

        def runahead_plan(c):
            """(skt -> list of next-super iters to emit) during super c."""
            if c + 1 >= NSUP:
                return {}
            nxt = ra_in[c + 1]
            plan = {}
            if c == 0:
                # gated on proj(3) (queries of super 1) at emission slot 9
                plan[10] = [0, 1, 2]
                for skt in range(11, 11 + nxt - 3):
                    plan[skt] = [skt - 8]
            else:
                fresh = list(range(ra_in[c], NSK))
                for j in range(nxt):
                    skt = fresh[min(len(fresh) - 1, (j * len(fresh)) // nxt)]
                    plan.setdefault(skt, []).append(j)
            return plan

        attn_state(0)
        next_piece = [0]
        for c in range(NSUP):
            rb = ra_in[c]
            plan = runahead_plan(c)
            pvq = 0
            fresh = list(range(rb, NSK))
            for idx, skt in enumerate(fresh):
                attn_sqexp(c, skt)
                # previous super's finalize lands after this super's first two
                # score emissions so rs/ib don't choke the psum ring and the
                # engines finalize in parallel with fresh work
                if c > 0 and idx == 1:
                    finalize_r(c - 1)
                    finalize_o(c - 1)
                # PV lags scores >= 1 iter (in-order PE stream must not head-
                # block on exp); super c's PVs wait for finalize_o(c-1) to
                # free the o accumulator; catch-up drains a few per iter.
                if c == 0 or idx >= 2:
                    budget = min(4, max(1, -(-(NSK - pvq) // max(1, NSK - skt))))
                    for _ in range(budget):
                        if pvq <= skt - 1:
                            attn_pv(c, pvq)
                            pvq += 1
                if c == 0:
                    target = min(len(piece_q), ((skt + 1) * 4 + 2) // 3)
                    while next_piece[0] < target:
                        piece_q[next_piece[0]]()
                        next_piece[0] += 1
                for na in plan.get(skt, ()):
                    if na == 0:
                        attn_state(c + 1)
                    attn_sqexp(c + 1, na)
            while pvq < NSK:
                attn_pv(c, pvq)
                pvq += 1
        finalize_r(NSUP - 1)
        finalize_o(NSUP - 1)

    nc.compile()
    return nc


_NC = None


def _get_nc():
    global _NC
    if _NC is None:
        _NC = _build()
    return _NC


def kernel(X, lam, Wq, bq, Wk, bk, Wv, bv):
    X = np.asarray(X, dtype=np.float32)
    lam_f = float(np.asarray(lam))
    Wq = np.asarray(Wq, np.float32)
    Wk = np.asarray(Wk, np.float32)
    Wv = np.asarray(Wv, np.float32)
    bq_a = np.asarray(bq, np.float32).reshape(2 * D, 1)
    bk_a = np.asarray(bk, np.float32).reshape(2 * D, 1)
    bv_a = np.asarray(bv, np.float32).reshape(D, 1).copy()
    wv16 = np.ascontiguousarray(Wv.astype(np.float16))

    nc = _get_nc()

    in_maps = []
    for core in range(8):
        b, i = divmod(core, 2)
        dsl = slice(i * D, (i + 1) * D)
        in_maps.append(
            {
                "xt": np.ascontiguousarray(X[b].T.astype(np.float16)),
                "wq": np.ascontiguousarray(Wq[:, dsl].astype(np.float16)),
                "wk": np.ascontiguousarray(Wk[:, dsl].astype(np.float16)),
                "wv": wv16,
                "bq": np.ascontiguousarray(bq_a[dsl]),
                "bk": np.ascontiguousarray(bk_a[dsl]),
                "bv": bv_a,
                "lam_row": np.full((1, 128), 1.0 if i == 0 else lam_f, np.float16),
            }
        )

    global LAST_RESULT
    kwargs = {}
    if TRACE:
        import tempfile

        tdir = tempfile.mkdtemp(dir=TRACE_DIR) if TRACE_DIR else None
        kwargs = dict(trace=True, tmpdir=tdir)
    res = run_bass_kernel_spmd(nc, in_maps, list(range(8)), **kwargs)
    LAST_RESULT = res

    o = np.empty((B, S, D), np.float32)
    for b in range(B):
        n0 = res.results[2 * b]["o"].astype(np.float32)
        n1 = res.results[2 * b + 1]["o"].astype(np.float32)
        o[b] = (n0 - n1).T
    return o


# revision 50
# speedup vs baseline: 1.0309x; 1.0283x over previous
"""DiffAttn kernel for 8 trn2 NeuronCores — v2 (component-split, fp16).

Problem (per reference):
  X [4, 4096, 1024]; Wq/Wk [1024, 256]; Wv [1024, 128]; biases; lam scalar.
  Q,K = X@Wq+bq, X@Wk+bk ; V = X@Wv+bv
  A_i = Q_i @ K_i^T / sqrt(128)  (i = 1,2 : the two 128-wide halves)
  out = (softmax(A1) - lam * softmax(A2)) @ V          -> [4, 4096, 128]

Sharding: 8 cores = 4 batches x 2 softmax components. Core (b, i) computes
n_i = softmax(A_i) @ V for ALL 4096 queries of batch b, normalized and (for
i=1) pre-scaled by lam via the broadcast matmul; the host computes
out[b] = n_0 - n_1 (pure elementwise post-processing, like the transpose).
This removes the K-projection redundancy of a query-split (each core
projects only its component's Q_i/K_i; V stays duplicated per pair).

All matmul data is fp16 (host converts X/W); PSUM accumulates fp32.
Per-core dataflow:
  XT [1024, 4096] fp16 streamed in 512-col chunks; Q_i/K_i projected into
  head-transposed layout [128, S]; V via DMA-xbar transpose into [key, d]
  blocks (no PE transpose, no PSUM). Bias-adds ride the PSUM->SBUF
  evacuation on VectorE (fp16 out). Scores S^T[sk, sq] per 1024-query
  super-chunk; exp on ScalarE (1/sqrt(D) folded into activation scale)
  writing fp16; softmax denominators accumulated on VectorE in fp16 (2x
  DVE mode); PV accumulated over the 32 key tiles in PSUM. Finalize:
  denominator row via fp16 ones-matmul column-sum, reciprocal via the
  fast custom-DVE approx (no ACT table switch), broadcast across
  partitions by a K=1 f32r matmul with lam folded into the stationary
  row, one fp16 multiply, ship O^T[128, 4096] fp16.

Emission interleaves the remaining projection chunks into super 0 and
runs the next super's scores/exp/denominator ahead (PV deferred until the
single o-PSUM accumulator frees) so ScalarE never starves while PE works
through projections.
"""

import os
import sys

sys.path.insert(0, "/opt/trn_rl_repo")

import numpy as np

import concourse.bacc as bacc
import concourse.mybir as mybir
from concourse.tile import TileContext
from concourse.bass_utils import run_bass_kernel_spmd

F32 = mybir.dt.float32
F16 = mybir.dt.float16
F32R = mybir.dt.float32r
AF = mybir.ActivationFunctionType

D = 128
EMB = 1024
B, S = 4, 4096
SQC = 512            # projection column chunk
NCC = S // SQC       # 8 chunks
NE = EMB // 128      # 8 emb tiles
SUP = 1024           # attention query super-chunk
NSUP = S // SUP      # 4
NSK = S // 128       # 32 key tiles
RA0 = 24             # run-ahead iters of super 1 emitted during super 0
RAL = 8              # boundary run-ahead for later supers
INV_SQRT_D = 1.0 / np.sqrt(np.float32(D))

TRACE = False
TRACE_DIR = None
LAST_RESULT = None


def _build():
    nc = bacc.Bacc("TRN2", target_bir_lowering=False, debug=False, num_devices=8)

    xt = nc.dram_tensor("xt", [EMB, S], F16, kind="ExternalInput")
    wq = nc.dram_tensor("wq", [EMB, D], F16, kind="ExternalInput")
    wk = nc.dram_tensor("wk", [EMB, D], F16, kind="ExternalInput")
    wv = nc.dram_tensor("wv", [EMB, D], F16, kind="ExternalInput")
    bq = nc.dram_tensor("bq", [D, 1], F32, kind="ExternalInput")
    bk = nc.dram_tensor("bk", [D, 1], F32, kind="ExternalInput")
    bv = nc.dram_tensor("bv", [D, 1], F32, kind="ExternalInput")
    lam_row = nc.dram_tensor("lam_row", [1, 128], F16, kind="ExternalInput")
    out = nc.dram_tensor("o", [D, S], F16, kind="ExternalOutput")  # O_i^T / r

    from contextlib import ExitStack

    with TileContext(nc) as tc, ExitStack() as ctx:
        xpool = ctx.enter_context(tc.tile_pool(name="xt", bufs=6))
        cpool = ctx.enter_context(tc.tile_pool(name="const", bufs=1))
        wpool = ctx.enter_context(tc.tile_pool(name="w", bufs=1))
        qkv = ctx.enter_context(tc.tile_pool(name="qkv", bufs=1))
        vpool = ctx.enter_context(tc.tile_pool(name="vts", bufs=2))
        epool = ctx.enter_context(tc.tile_pool(name="e", bufs=28))
        apool = ctx.enter_context(tc.tile_pool(name="acc", bufs=3))
        fpool = ctx.enter_context(tc.tile_pool(name="fin", bufs=2))
        smpool = ctx.enter_context(tc.tile_pool(name="small", bufs=2))
        pmm = ctx.enter_context(tc.tile_pool(name="pmm", bufs=1, space="PSUM"))

        # constants / small inputs
        ones_col = cpool.tile([128, 1], F16, tag="ones_col")
        nc.vector.memset(ones_col[:], 1.0)
        jsrc = cpool.tile([128, 512], F16, tag="jsrc")
        nc.vector.memset(jsrc[:], 0.0)
        from concourse import masks

        ident = cpool.tile([128, 128], F16, tag="ident")
        masks.make_identity(nc, ident[:])
        bq_t = cpool.tile([128, 1], F32, tag="bq")
        bk_t = cpool.tile([128, 1], F32, tag="bk")
        bv_t = cpool.tile([128, 1], F32, tag="bv")
        lam_t = cpool.tile([1, 128], F16, tag="lam")
        nc.gpsimd.dma_start(out=bq_t[:], in_=bq[:, :])
        nc.gpsimd.dma_start(out=bk_t[:], in_=bk[:, :])
        nc.gpsimd.dma_start(out=bv_t[:], in_=bv[:, :])
        nc.gpsimd.dma_start(out=lam_t[:], in_=lam_row[:, :])

        # weights [128, NE, 128] fp16 — one DMA each (issue slots are precious)
        wq_t = wpool.tile([128, NE, 128], F16, tag="wq")
        wk_t = wpool.tile([128, NE, 128], F16, tag="wk")
        wv_t = wpool.tile([128, NE, 128], F16, tag="wv")

        def wsrc(w):
            return w[:, :].rearrange("(t p) d -> p t d", p=128)



        xts = {}

        def load_chunk(cc, sliced=False):
            # chunks split across the two HWDGE queues; prologue chunks are
            # e-tile-sliced so projections can start on slice 0 (subtile deps)
            if cc in xts or cc >= NCC:
                return
            t = xpool.tile([128, NE, SQC], F16, tag="xchunk", name=f"xc_{cc}")
            c0 = cc * SQC
            if sliced:
                for j, eng in ((0, nc.scalar), (1, nc.sync), (2, nc.scalar), (3, nc.sync)):
                    r = slice(j * 256, (j + 1) * 256)
                    eng.dma_start(
                        out=t[:, 2 * j : 2 * j + 2, :],
                        in_=xt[r, c0 : c0 + SQC].rearrange(
                            "(t p) s -> p t s", p=128
                        ),
                    )
            else:
                h = SQC // 2
                nc.scalar.dma_start(
                    out=t[:, :, 0:h],
                    in_=xt[:, c0 : c0 + h].rearrange("(t p) s -> p t s", p=128),
                )
                nc.sync.dma_start(
                    out=t[:, :, h:SQC],
                    in_=xt[:, c0 + h : c0 + SQC].rearrange("(t p) s -> p t s", p=128),
                )
            xts[cc] = t

        # projected tensors
        qt = qkv.tile([128, S], F16, tag="qt")   # Q_i^T
        kt = qkv.tile([128, S], F16, tag="kt")   # K_i^T
        vv = qkv.tile([128, S], F16, tag="vv")   # col blk*128+j : V[key, d]

        # ---------------- projection of one chunk ----------------
        # emitted in half-group pieces (4 matmuls each) so the in-order PE
        # stream never starves ScalarE for a whole chunk
        pend_tr = []

        def proj_pieces(cc):
            csl = slice(cc * SQC, (cc + 1) * SQC)
            state = {}
            pieces = []

            def mk(dst, w_t, b_t, gtag):
                def p1():
                    ps = pmm.tile(
                        [128, SQC], F32, tag="s", bufs=3, name=f"pp_{gtag}_{cc}"
                    )
                    state[gtag] = ps
                    for t in range(NE // 2):
                        nc.tensor.matmul(
                            ps[:], w_t[:, t, :], xts[cc][:, t, :],
                            start=(t == 0), stop=False,
                        )

                def p2():
                    ps = state[gtag]
                    for t in range(NE // 2, NE):
                        nc.tensor.matmul(
                            ps[:], w_t[:, t, :], xts[cc][:, t, :],
                            start=False, stop=(t == NE - 1),
                        )
                    if dst is not None:
                        nc.vector.tensor_scalar_add(dst[:, csl], ps[:], b_t[:, 0:1])
                    else:
                        vt_s = vpool.tile(
                            [128, SQC], F16, tag="vts", name=f"vts_{cc}"
                        )
                        nc.vector.tensor_scalar_add(vt_s[:], ps[:], b_t[:, 0:1])
                        for j in range(SQC // 128):
                            col = (cc * (SQC // 128) + j) * 128
                            tr = pmm.tile(
                                [128, 128], F16, tag="s", bufs=3,
                                name=f"tr_{cc}_{j}",
                            )
                            nc.tensor.transpose(
                                tr[:], vt_s[:, j * 128 : (j + 1) * 128], ident[:]
                            )
                            nc.vector.tensor_copy(vv[:, col : col + 128], tr[:])

                pieces.extend([p1, p2])

            mk(kt, wk_t, bk_t, "k")
            mk(qt, wq_t, bq_t, "q")
            mk(None, wv_t, bv_t, "v")
            return pieces

        def proj(cc):
            for p in proj_pieces(cc):
                p()

        # ---------------- attention ----------------
        st = {}

        def attn_state(c):
            st[c] = dict(
                o=None,
                pacc=apool.tile([128, SUP], F16, tag="pacc", name=f"pacc_{c}"),
                e={},
            )

        def attn_sqexp(c, skt):
            """scores + exp + denominator-accumulate for (c, skt)."""
            s = st[c]
            ksl = slice(skt * 128, (skt + 1) * 128)
            s_ps = pmm.tile([128, SUP], F32, tag="s", bufs=3, name=f"s_{c}_{skt}")
            for h in range(2):
                hsl = slice(h * 512, (h + 1) * 512)
                nc.tensor.matmul(
                    s_ps[:, hsl], kt[:, ksl],
                    qt[:, c * SUP + h * 512 : c * SUP + (h + 1) * 512],
                    start=True, stop=True,
                )
            e_t = epool.tile([128, SUP], F16, tag="e", name=f"e_{c}_{skt}")
            nc.scalar.activation(e_t[:], s_ps[:], AF.Exp, scale=float(INV_SQRT_D))
            pacc = s["pacc"]
            if skt == 0:
                nc.vector.tensor_copy(pacc[:], e_t[:])
            else:
                nc.vector.tensor_add(pacc[:], pacc[:], e_t[:])
            s["e"][skt] = e_t

        def attn_pv(c, skt):
            s = st[c]
            if s["o"] is None:
                s["o"] = pmm.tile([128, SUP], F32, tag="o", bufs=1, name=f"o_{c}")
            ksl = slice(skt * 128, (skt + 1) * 128)
            e_t = s["e"].pop(skt)
            for h in range(2):
                hsl = slice(h * 512, (h + 1) * 512)
                nc.tensor.matmul(
                    s["o"][:, hsl], vv[:, ksl], e_t[:, hsl],
                    start=(skt == 0), stop=(skt == NSK - 1),
                )

        fin_ib = {}

        def finalize_r(c):
            """Denominator chain — depends only on pacc, overlaps the PV drain."""
            s = st[c]
            # evacuate O first (DVE): frees the o psum for the next super's
            # catch-up PVs without stealing ScalarE exp time
            o_s = fpool.tile([128, SUP], F16, tag="os", name=f"os_{c}")
            nc.vector.tensor_copy(o_s[:], s["o"][:])
            s["o_s"] = o_s
            rs = pmm.tile([1, SUP], F32, tag="s", bufs=3, name=f"rs_{c}")
            for h in range(2):
                hsl = slice(h * 512, (h + 1) * 512)
                nc.tensor.matmul(
                    rs[0:1, hsl], ones_col[:], s["pacc"][:, hsl],
                    start=True, stop=True,
                )
            r_inv = smpool.tile([1, SUP], F32, tag="rinv", name=f"rinv_{c}")
            nc.vector.reciprocal_approx_fast(r_inv[0:1, :], rs[0:1, :])
            r16 = smpool.tile([1, SUP], F16, tag="r16", name=f"r16_{c}")
            nc.vector.tensor_copy(r16[0:1, :], r_inv[0:1, :])
            # broadcast lam/r across partitions (lam folded into stationary row)
            ib = pmm.tile([128, SUP], F32, tag="s", bufs=3, name=f"ib_{c}")
            for h in range(2):
                hsl = slice(h * 512, (h + 1) * 512)
                nc.tensor.matmul(
                    ib[:, hsl], lam_t[0:1, :], r16[0:1, hsl],
                    start=True, stop=True,
                )
            fin_ib[c] = ib

        def finalize_o(c):
            s = st.pop(c)
            o_t = fpool.tile([128, SUP], F16, tag="ot", name=f"ot_{c}")
            nc.vector.tensor_mul(o_t[:], s["o_s"][:], fin_ib.pop(c)[:])
            nc.sync.dma_start(out=out[:, c * SUP : (c + 1) * SUP], in_=o_t[:])

        # ---------------- emission schedule ----------------
        # prologue DMA order tuned so proj(0)'s first matmul has wk slice 0 +
        # chunk-0 slice 0 within ~2us of queue start
        nc.sync.dma_start(out=wk_t[:, 0:4, :], in_=wk[0:512, :].rearrange("(t p) d -> p t d", p=128))
        load_chunk(0, sliced=True)
        nc.sync.dma_start(out=wk_t[:, 4:8, :], in_=wk[512:1024, :].rearrange("(t p) d -> p t d", p=128))
        nc.scalar.dma_start(out=wq_t[:], in_=wsrc(wq))
        nc.sync.dma_start(out=wv_t[:], in_=wsrc(wv))
        load_chunk(1, sliced=True)
        load_chunk(2)
        load_chunk(3)
        load_chunk(4)
        load_chunk(5)
        # junk matmuls against the HAM clock gate: keep the PE busy while the
        # first chunk streams in so projections start at 2.4 GHz
        junk = pmm.tile([1, 512], F32, tag="o", bufs=1, name="junk")
        for _ in range(6):
            nc.tensor.matmul(junk[0:1, :], ones_col[:], jsrc[:], start=True, stop=True)
        proj(0)
        load_chunk(6)
        proj(1)
        load_chunk(7)
        # remaining proj chunks drip into super 0 as half-group pieces;
        # chunk cc must complete before attn(0, 4*cc)
        piece_q = []
        for cc in range(2, NCC):
            piece_q.extend(proj_pieces(cc))

        # ra_in[c]: iterations of super c pre-run during super c-1.  Super 0's
        # window is PE-bound (projections), so most of super 1 pre-runs there
        # to keep ScalarE saturated; later boundaries only need a small lap.
        ra_in = [0, RA0, RAL, RAL]

        def runahead_plan(c):
            """(skt -> list of next-super iters to emit) during super c."""
            if c + 1 >= NSUP:
                return {}
            nxt = ra_in[c + 1]
            plan = {}
            if c == 0:
                # gated on proj(3) (queries of super 1) at emission slot 9
                plan[10] = [0, 1, 2]
                for skt in range(11, 11 + nxt - 3):
                    plan[skt] = [skt - 8]
            else:
                fresh = list(range(ra_in[c], NSK))
                for j in range(nxt):
                    skt = fresh[min(len(fresh) - 1, (j * len(fresh)) // nxt)]
                    plan.setdefault(skt, []).append(j)
            return plan

        attn_state(0)
        next_piece = [0]
        for c in range(NSUP):
            rb = ra_in[c]
            plan = runahead_plan(c)
            pvq = 0
            fresh = list(range(rb, NSK))
            for idx, skt in enumerate(fresh):
                attn_sqexp(c, skt)
                # previous super's finalize lands after this super's first two
                # score emissions so rs/ib don't choke the psum ring and the
                # engines finalize in parallel with fresh work
                if c > 0 and idx == 1:
                    finalize_r(c - 1)
                    finalize_o(c - 1)
                # PV lags scores >= 1 iter (in-order PE stream must not head-
                # block on exp); super c's PVs wait for finalize_o(c-1) to
                # free the o accumulator; catch-up drains a few per iter.
                if c == 0 or idx >= 2:
                    budget = min(4, max(1, -(-(NSK - pvq) // max(1, NSK - skt))))
                    for _ in range(budget):
                        if pvq <= skt - 1:
                            attn_pv(c, pvq)
                            pvq += 1
                if c == 0:
                    target = min(len(piece_q), ((skt + 1) * 4 + 2) // 3)
                    while next_piece[0] < target:
                        piece_q[next_piece[0]]()
                        next_piece[0] += 1
                for na in plan.get(skt, ()):
                    if na == 0:
                        attn_state(c + 1)
                    attn_sqexp(c + 1, na)
            while pvq < NSK:
                attn_pv(c, pvq)
                pvq += 1
        finalize_r(NSUP - 1)
        finalize_o(NSUP - 1)

    nc.compile()
    return nc


_NC = None


def _get_nc():
    global _NC
    if _NC is None:
        _NC = _build()
    return _NC


def kernel(X, lam, Wq, bq, Wk, bk, Wv, bv):
    X = np.asarray(X, dtype=np.float32)
    lam_f = float(np.asarray(lam))
    Wq = np.asarray(Wq, np.float32)
    Wk = np.asarray(Wk, np.float32)
    Wv = np.asarray(Wv, np.float32)
    bq_a = np.asarray(bq, np.float32).reshape(2 * D, 1)
    bk_a = np.asarray(bk, np.float32).reshape(2 * D, 1)
    bv_a = np.asarray(bv, np.float32).reshape(D, 1).copy()
    wv16 = np.ascontiguousarray(Wv.astype(np.float16))

    nc = _get_nc()

    in_maps = []
    for core in range(8):
        b, i = divmod(core, 2)
        dsl = slice(i * D, (i + 1) * D)
        in_maps.append(
            {
                "xt": np.ascontiguousarray(X[b].T.astype(np.float16)),
                "wq": np.ascontiguousarray(Wq[:, dsl].astype(np.float16)),
                "wk": np.ascontiguousarray(Wk[:, dsl].astype(np.float16)),
                "wv": wv16,
                "bq": np.ascontiguousarray(bq_a[dsl]),
                "bk": np.ascontiguousarray(bk_a[dsl]),
                "bv": bv_a,
                "lam_row": np.full((1, 128), 1.0 if i == 0 else lam_f, np.float16),
            }
        )

    global LAST_RESULT
    kwargs = {}
    if TRACE:
        import tempfile

        tdir = tempfile.mkdtemp(dir=TRACE_DIR) if TRACE_DIR else None
        kwargs = dict(trace=True, tmpdir=tdir)
    res = run_bass_kernel_spmd(nc, in_maps, list(range(8)), **kwargs)
    LAST_RESULT = res

    o = np.empty((B, S, D), np.float32)
    for b in range(B):
        n0 = res.results[2 * b]["o"].astype(np.float32)
        n1 = res.results[2 * b + 1]["o"].astype(np.float32)
        o[b] = (n0 - n1).T
    return o


# revision 51
# speedup vs baseline: 1.0350x; 1.0040x over previous
"""DiffAttn kernel for 8 trn2 NeuronCores — v2 (component-split, fp16).

Problem (per reference):
  X [4, 4096, 1024]; Wq/Wk [1024, 256]; Wv [1024, 128]; biases; lam scalar.
  Q,K = X@Wq+bq, X@Wk+bk ; V = X@Wv+bv
  A_i = Q_i @ K_i^T / sqrt(128)  (i = 1,2 : the two 128-wide halves)
  out = (softmax(A1) - lam * softmax(A2)) @ V          -> [4, 4096, 128]

Sharding: 8 cores = 4 batches x 2 softmax components. Core (b, i) computes
n_i = softmax(A_i) @ V for ALL 4096 queries of batch b, normalized and (for
i=1) pre-scaled by lam via the broadcast matmul; the host computes
out[b] = n_0 - n_1 (pure elementwise post-processing, like the transpose).
This removes the K-projection redundancy of a query-split (each core
projects only its component's Q_i/K_i; V stays duplicated per pair).

All matmul data is fp16 (host converts X/W); PSUM accumulates fp32.
Per-core dataflow:
  XT [1024, 4096] fp16 streamed in 512-col chunks over both HWDGE queues
  (first two chunks e-tile-sliced so projections start on slice 0);
  Q_i/K_i projected into head-transposed layout [128, S]; V via PE
  transpose into [key, d] blocks. Bias-adds ride the PSUM->SBUF
  evacuation on VectorE (fp16 out). Scores S^T[sk, sq] per 1024-query
  super-chunk; exp on ScalarE (1/sqrt(D) folded into activation scale)
  writing fp16; softmax denominators accumulated on VectorE in fp16 (2x
  DVE mode); PV accumulated over the 32 key tiles in PSUM. Finalize:
  O evacuated first (frees the single o-PSUM accumulator), denominator
  row via fp16 ones-matmul column-sum, reciprocal via the fast
  custom-DVE approx (no ACT table switch), broadcast across partitions
  by a K=1 fp16 matmul with lam folded into the stationary row, one
  fp16 multiply, ship O^T[128, 4096] fp16.

Because every engine executes its stream in order, emission order IS the
schedule: PV lags scores by one iteration (no head-blocking on exp),
projections drip into super 0 as 4-matmul pieces, most of super 1 is
pre-run during PE-bound super 0, later boundaries pre-run a small lap,
and each super's finalize lands inside the next super's stream. A few
junk matmuls warm the HAM clock gate during the prologue DMA wait.
"""

import os
import sys

sys.path.insert(0, "/opt/trn_rl_repo")

import numpy as np

import concourse.bacc as bacc
import concourse.mybir as mybir
from concourse.tile import TileContext
from concourse.bass_utils import run_bass_kernel_spmd

F32 = mybir.dt.float32
F16 = mybir.dt.float16
F32R = mybir.dt.float32r
AF = mybir.ActivationFunctionType

D = 128
EMB = 1024
B, S = 4, 4096
SQC = 512            # projection column chunk
NCC = S // SQC       # 8 chunks
NE = EMB // 128      # 8 emb tiles
SUP = 1024           # attention query super-chunk
NSUP = S // SUP      # 4
NSK = S // 128       # 32 key tiles
RA0 = 24             # run-ahead iters of super 1 emitted during super 0
RAL = 8              # boundary run-ahead for later supers
INV_SQRT_D = 1.0 / np.sqrt(np.float32(D))

TRACE = False
TRACE_DIR = None
LAST_RESULT = None


def _build():
    nc = bacc.Bacc("TRN2", target_bir_lowering=False, debug=False, num_devices=8)

    xt = nc.dram_tensor("xt", [EMB, S], F16, kind="ExternalInput")
    wq = nc.dram_tensor("wq", [EMB, D], F16, kind="ExternalInput")
    wk = nc.dram_tensor("wk", [EMB, D], F16, kind="ExternalInput")
    wv = nc.dram_tensor("wv", [EMB, D], F16, kind="ExternalInput")
    bq = nc.dram_tensor("bq", [D, 1], F32, kind="ExternalInput")
    bk = nc.dram_tensor("bk", [D, 1], F32, kind="ExternalInput")
    bv = nc.dram_tensor("bv", [D, 1], F32, kind="ExternalInput")
    lam_row = nc.dram_tensor("lam_row", [1, 128], F16, kind="ExternalInput")
    out = nc.dram_tensor("o", [D, S], F16, kind="ExternalOutput")  # O_i^T / r

    from contextlib import ExitStack

    with TileContext(nc) as tc, ExitStack() as ctx:
        xpool = ctx.enter_context(tc.tile_pool(name="xt", bufs=6))
        cpool = ctx.enter_context(tc.tile_pool(name="const", bufs=1))
        wpool = ctx.enter_context(tc.tile_pool(name="w", bufs=1))
        qkv = ctx.enter_context(tc.tile_pool(name="qkv", bufs=1))
        vpool = ctx.enter_context(tc.tile_pool(name="vts", bufs=2))
        epool = ctx.enter_context(tc.tile_pool(name="e", bufs=28))
        apool = ctx.enter_context(tc.tile_pool(name="acc", bufs=3))
        fpool = ctx.enter_context(tc.tile_pool(name="fin", bufs=2))
        smpool = ctx.enter_context(tc.tile_pool(name="small", bufs=2))
        pmm = ctx.enter_context(tc.tile_pool(name="pmm", bufs=1, space="PSUM"))

        # constants / small inputs
        ones_col = cpool.tile([128, 1], F16, tag="ones_col")
        nc.vector.memset(ones_col[:], 1.0)
        jsrc = cpool.tile([128, 512], F16, tag="jsrc")
        nc.vector.memset(jsrc[:], 0.0)
        from concourse import masks

        ident = cpool.tile([128, 128], F16, tag="ident")
        masks.make_identity(nc, ident[:])
        bq_t = cpool.tile([128, 1], F32, tag="bq")
        bk_t = cpool.tile([128, 1], F32, tag="bk")
        bv_t = cpool.tile([128, 1], F32, tag="bv")
        lam_t = cpool.tile([1, 128], F16, tag="lam")
        nc.gpsimd.dma_start(out=bq_t[:], in_=bq[:, :])
        nc.gpsimd.dma_start(out=bk_t[:], in_=bk[:, :])
        nc.gpsimd.dma_start(out=bv_t[:], in_=bv[:, :])
        nc.gpsimd.dma_start(out=lam_t[:], in_=lam_row[:, :])

        # weights [128, NE, 128] fp16 — one DMA each (issue slots are precious)
        wq_t = wpool.tile([128, NE, 128], F16, tag="wq")
        wk_t = wpool.tile([128, NE, 128], F16, tag="wk")
        wv_t = wpool.tile([128, NE, 128], F16, tag="wv")

        def wsrc(w):
            return w[:, :].rearrange("(t p) d -> p t d", p=128)



        xts = {}

        def load_chunk(cc, sliced=False):
            # chunks split across the two HWDGE queues; prologue chunks are
            # e-tile-sliced so projections can start on slice 0 (subtile deps)
            if cc in xts or cc >= NCC:
                return
            t = xpool.tile([128, NE, SQC], F16, tag="xchunk", name=f"xc_{cc}")
            c0 = cc * SQC
            if sliced:
                for j, eng in ((0, nc.scalar), (1, nc.sync), (2, nc.scalar), (3, nc.sync)):
                    r = slice(j * 256, (j + 1) * 256)
                    eng.dma_start(
                        out=t[:, 2 * j : 2 * j + 2, :],
                        in_=xt[r, c0 : c0 + SQC].rearrange(
                            "(t p) s -> p t s", p=128
                        ),
                    )
            else:
                h = SQC // 2
                nc.scalar.dma_start(
                    out=t[:, :, 0:h],
                    in_=xt[:, c0 : c0 + h].rearrange("(t p) s -> p t s", p=128),
                )
                nc.sync.dma_start(
                    out=t[:, :, h:SQC],
                    in_=xt[:, c0 + h : c0 + SQC].rearrange("(t p) s -> p t s", p=128),
                )
            xts[cc] = t

        # projected tensors
        qt = qkv.tile([128, S], F16, tag="qt")   # Q_i^T
        kt = qkv.tile([128, S], F16, tag="kt")   # K_i^T
        vv = qkv.tile([128, S], F16, tag="vv")   # col blk*128+j : V[key, d]

        # ---------------- projection of one chunk ----------------
        # emitted in half-group pieces (4 matmuls each) so the in-order PE
        # stream never starves ScalarE for a whole chunk
        pend_tr = []

        def proj_pieces(cc):
            csl = slice(cc * SQC, (cc + 1) * SQC)
            state = {}
            pieces = []

            def mk(dst, w_t, b_t, gtag):
                def p1():
                    ps = pmm.tile(
                        [128, SQC], F32, tag="s", bufs=3, name=f"pp_{gtag}_{cc}"
                    )
                    state[gtag] = ps
                    for t in range(NE // 2):
                        nc.tensor.matmul(
                            ps[:], w_t[:, t, :], xts[cc][:, t, :],
                            start=(t == 0), stop=False,
                        )

                def p2():
                    ps = state[gtag]
                    for t in range(NE // 2, NE):
                        nc.tensor.matmul(
                            ps[:], w_t[:, t, :], xts[cc][:, t, :],
                            start=False, stop=(t == NE - 1),
                        )
                    if dst is not None:
                        nc.vector.tensor_scalar_add(dst[:, csl], ps[:], b_t[:, 0:1])
                    else:
                        vt_s = vpool.tile(
                            [128, SQC], F16, tag="vts", name=f"vts_{cc}"
                        )
                        nc.vector.tensor_scalar_add(vt_s[:], ps[:], b_t[:, 0:1])
                        for j in range(SQC // 128):
                            col = (cc * (SQC // 128) + j) * 128
                            tr = pmm.tile(
                                [128, 128], F16, tag="s", bufs=3,
                                name=f"tr_{cc}_{j}",
                            )
                            nc.tensor.transpose(
                                tr[:], vt_s[:, j * 128 : (j + 1) * 128], ident[:]
                            )
                            nc.vector.tensor_copy(vv[:, col : col + 128], tr[:])

                pieces.extend([p1, p2])

            mk(kt, wk_t, bk_t, "k")
            mk(qt, wq_t, bq_t, "q")
            mk(None, wv_t, bv_t, "v")
            return pieces

        def proj(cc):
            for p in proj_pieces(cc):
                p()

        # ---------------- attention ----------------
        st = {}

        def attn_state(c):
            st[c] = dict(
                o=None,
                pacc=apool.tile([128, SUP], F16, tag="pacc", name=f"pacc_{c}"),
                e={},
            )

        def attn_sqexp(c, skt):
            """scores + exp + denominator-accumulate for (c, skt)."""
            s = st[c]
            ksl = slice(skt * 128, (skt + 1) * 128)
            s_ps = pmm.tile([128, SUP], F32, tag="s", bufs=3, name=f"s_{c}_{skt}")
            for h in range(2):
                hsl = slice(h * 512, (h + 1) * 512)
                nc.tensor.matmul(
                    s_ps[:, hsl], kt[:, ksl],
                    qt[:, c * SUP + h * 512 : c * SUP + (h + 1) * 512],
                    start=True, stop=True,
                )
            e_t = epool.tile([128, SUP], F16, tag="e", name=f"e_{c}_{skt}")
            nc.scalar.activation(e_t[:], s_ps[:], AF.Exp, scale=float(INV_SQRT_D))
            pacc = s["pacc"]
            if skt == 0:
                nc.vector.tensor_copy(pacc[:], e_t[:])
            else:
                nc.vector.tensor_add(pacc[:], pacc[:], e_t[:])
            s["e"][skt] = e_t

        def attn_pv(c, skt):
            s = st[c]
            if s["o"] is None:
                s["o"] = pmm.tile([128, SUP], F32, tag="o", bufs=1, name=f"o_{c}")
            ksl = slice(skt * 128, (skt + 1) * 128)
            e_t = s["e"].pop(skt)
            for h in range(2):
                hsl = slice(h * 512, (h + 1) * 512)
                nc.tensor.matmul(
                    s["o"][:, hsl], vv[:, ksl], e_t[:, hsl],
                    start=(skt == 0), stop=(skt == NSK - 1),
                )

        fin_ib = {}

        def finalize_r(c):
            """Denominator chain — depends only on pacc, overlaps the PV drain."""
            s = st[c]
            # evacuate O first (DVE): frees the o psum for the next super's
            # catch-up PVs without stealing ScalarE exp time
            o_s = fpool.tile([128, SUP], F16, tag="os", name=f"os_{c}")
            nc.vector.tensor_copy(o_s[:], s["o"][:])
            s["o_s"] = o_s
            rs = pmm.tile([1, SUP], F32, tag="s", bufs=3, name=f"rs_{c}")
            for h in range(2):
                hsl = slice(h * 512, (h + 1) * 512)
                nc.tensor.matmul(
                    rs[0:1, hsl], ones_col[:], s["pacc"][:, hsl],
                    start=True, stop=True,
                )
            r_inv = smpool.tile([1, SUP], F32, tag="rinv", name=f"rinv_{c}")
            nc.vector.reciprocal_approx_fast(r_inv[0:1, :], rs[0:1, :])
            r16 = smpool.tile([1, SUP], F16, tag="r16", name=f"r16_{c}")
            nc.vector.tensor_copy(r16[0:1, :], r_inv[0:1, :])
            # broadcast lam/r across partitions (lam folded into stationary row)
            ib = pmm.tile([128, SUP], F32, tag="s", bufs=3, name=f"ib_{c}")
            for h in range(2):
                hsl = slice(h * 512, (h + 1) * 512)
                nc.tensor.matmul(
                    ib[:, hsl], lam_t[0:1, :], r16[0:1, hsl],
                    start=True, stop=True,
                )
            fin_ib[c] = ib

        def finalize_o(c):
            s = st.pop(c)
            o_t = fpool.tile([128, SUP], F16, tag="ot", name=f"ot_{c}")
            nc.vector.tensor_mul(o_t[:], s["o_s"][:], fin_ib.pop(c)[:])
            nc.sync.dma_start(out=out[:, c * SUP : (c + 1) * SUP], in_=o_t[:])

        # ---------------- emission schedule ----------------
        # prologue DMA order tuned so proj(0)'s first matmul has wk slice 0 +
        # chunk-0 slice 0 within ~2us of queue start
        nc.sync.dma_start(out=wk_t[:, 0:4, :], in_=wk[0:512, :].rearrange("(t p) d -> p t d", p=128))
        load_chunk(0, sliced=True)
        nc.sync.dma_start(out=wk_t[:, 4:8, :], in_=wk[512:1024, :].rearrange("(t p) d -> p t d", p=128))
        nc.scalar.dma_start(out=wq_t[:], in_=wsrc(wq))
        nc.sync.dma_start(out=wv_t[:], in_=wsrc(wv))
        load_chunk(1, sliced=True)
        load_chunk(2)
        load_chunk(3)
        load_chunk(4)
        load_chunk(5)
        # junk matmuls against the HAM clock gate: keep the PE busy while the
        # first chunk streams in so projections start at 2.4 GHz
        junk = pmm.tile([1, 512], F32, tag="o", bufs=1, name="junk")
        for _ in range(6):
            nc.tensor.matmul(junk[0:1, :], ones_col[:], jsrc[:], start=True, stop=True)
        proj(0)
        load_chunk(6)
        proj(1)
        load_chunk(7)
        # remaining proj chunks drip into super 0 as half-group pieces;
        # chunk cc must complete before attn(0, 4*cc)
        piece_q = []
        for cc in range(2, NCC):
            piece_q.extend(proj_pieces(cc))

        # ra_in[c]: iterations of super c pre-run during super c-1.  Super 0's
        # window is PE-bound (projections), so most of super 1 pre-runs there
        # to keep ScalarE saturated; later boundaries only need a small lap.
        ra_in = [0, RA0, RAL, RAL]

        def runahead_plan(c):
            """(skt -> list of next-super iters to emit) during super c."""
            if c + 1 >= NSUP:
                return {}
            nxt = ra_in[c + 1]
            plan = {}
            if c == 0:
                # gated on proj(3) (queries of super 1) at emission slot 9
                plan[10] = [0, 1, 2]
                for skt in range(11, 11 + nxt - 3):
                    plan[skt] = [skt - 8]
            else:
                fresh = list(range(ra_in[c], NSK))
                for j in range(nxt):
                    skt = fresh[min(len(fresh) - 1, (j * len(fresh)) // nxt)]
                    plan.setdefault(skt, []).append(j)
            return plan

        attn_state(0)
        next_piece = [0]
        for c in range(NSUP):
            rb = ra_in[c]
            plan = runahead_plan(c)
            pvq = 0
            fresh = list(range(rb, NSK))
            for idx, skt in enumerate(fresh):
                attn_sqexp(c, skt)
                # previous super's finalize lands after this super's first two
                # score emissions so rs/ib don't choke the psum ring and the
                # engines finalize in parallel with fresh work
                if c > 0 and idx == 1:
                    finalize_r(c - 1)
                    finalize_o(c - 1)
                # PV lags scores >= 1 iter (in-order PE stream must not head-
                # block on exp); super c's PVs wait for finalize_o(c-1) to
                # free the o accumulator; catch-up drains a few per iter.
                if c == 0 or idx >= 2:
                    budget = min(4, max(1, -(-(NSK - pvq) // max(1, NSK - skt))))
                    for _ in range(budget):
                        if pvq <= skt - 1:
                            attn_pv(c, pvq)
                            pvq += 1
                if c == 0:
                    target = min(len(piece_q), ((skt + 1) * 4 + 2) // 3)
                    while next_piece[0] < target:
                        piece_q[next_piece[0]]()
                        next_piece[0] += 1
                for na in plan.get(skt, ()):
                    if na == 0:
                        attn_state(c + 1)
                    attn_sqexp(c + 1, na)
            while pvq < NSK:
                attn_pv(c, pvq)
                pvq += 1
        finalize_r(NSUP - 1)
        finalize_o(NSUP - 1)

    nc.compile()
    return nc


_NC = None


def _get_nc():
    global _NC
    if _NC is None:
        _NC = _build()
    return _NC


def kernel(X, lam, Wq, bq, Wk, bk, Wv, bv):
    X = np.asarray(X, dtype=np.float32)
    lam_f = float(np.asarray(lam))
    Wq = np.asarray(Wq, np.float32)
    Wk = np.asarray(Wk, np.float32)
    Wv = np.asarray(Wv, np.float32)
    bq_a = np.asarray(bq, np.float32).reshape(2 * D, 1)
    bk_a = np.asarray(bk, np.float32).reshape(2 * D, 1)
    bv_a = np.asarray(bv, np.float32).reshape(D, 1).copy()
    wv16 = np.ascontiguousarray(Wv.astype(np.float16))

    nc = _get_nc()

    in_maps = []
    for core in range(8):
        b, i = divmod(core, 2)
        dsl = slice(i * D, (i + 1) * D)
        in_maps.append(
            {
                "xt": np.ascontiguousarray(X[b].T.astype(np.float16)),
                "wq": np.ascontiguousarray(Wq[:, dsl].astype(np.float16)),
                "wk": np.ascontiguousarray(Wk[:, dsl].astype(np.float16)),
                "wv": wv16,
                "bq": np.ascontiguousarray(bq_a[dsl]),
                "bk": np.ascontiguousarray(bk_a[dsl]),
                "bv": bv_a,
                "lam_row": np.full((1, 128), 1.0 if i == 0 else lam_f, np.float16),
            }
        )

    global LAST_RESULT
    kwargs = {}
    if TRACE:
        import tempfile

        tdir = tempfile.mkdtemp(dir=TRACE_DIR) if TRACE_DIR else None
        kwargs = dict(trace=True, tmpdir=tdir)
    res = run_bass_kernel_spmd(nc, in_maps, list(range(8)), **kwargs)
    LAST_RESULT = res

    o = np.empty((B, S, D), np.float32)
    for b in range(B):
        n0 = res.results[2 * b]["o"].astype(np.float32)
        n1 = res.results[2 * b + 1]["o"].astype(np.float32)
        o[b] = (n0 - n1).T
    return o
